# revision 5
# baseline (speedup 1.0000x reference)
"""Trainium2 Bass kernel for nn_AttentionBlock (B=4, H=W=64, C=256), SPMD over 8 NeuronCores.

Strategy:
  - Shard: batch b = core//2, query-half = core%2 (2048 queries/core, full 4096 keys).
    Key order is permuted per-core (own-half first) — softmax is permutation-invariant.
  - All matmuls in float32r (tf32): host pre-rounds inputs; PE multiplies exactly with
    fp32 PSUM accumulation (verified: only error source is the tf32 input rounding).
  - Transposed dataflow: K^T/Q^T [c, pos]; scores computed as S^T [kk, q] so softmax's
    key-reduction becomes a PE ones-matmul; exp via ACT with per-partition bias
    tau[kk] = <bq/16, K[kk]> (the only part of the q/k biases softmax doesn't cancel).
  - V natural layout [kk, c] feeds attn@V directly: O'^T[c,q] accumulated over 32 kk tiles.
  - Device returns Y^T = wp.T @ O'^T (unnormalized) + denominators; host divides,
    adds bv@wp + bp and the residual x.
"""
import numpy as np

B, HH, WW, C = 4, 64, 64, 256
HW = HH * WW          # 4096 spatial positions
QH = HW // 2          # 2048 queries per core
NC = 8
KT = HW // 128        # 32 kk tiles
QCH = QH // 512       # 4 query chunks of 512
# blob columns: xT | wq/16 | wk | wv | wp | bq/16
XO, WQO, WKO, WVO, WPO, BQO = 0, HW, HW + C, HW + 2 * C, HW + 3 * C, HW + 4 * C
WBLOB = HW + 4 * C + 2  # 5122 (bq stored twice: fp32r matmul needs even N)


def tf32_round(x: np.ndarray) -> np.ndarray:
    u = np.ascontiguousarray(x, np.float32).view(np.uint32).astype(np.uint64)
    u = (u + 0x1000 + ((u >> 13) & 1)) & 0xFFFFE000
    return u.astype(np.uint32).view(np.float32)


def build_nc(bench_iters=None):
    import contextlib
    import concourse.bass as bass  # noqa: F401
    import concourse.tile as tile
    from concourse import bacc, mybir
    from concourse import bass_isa

    f32 = mybir.dt.float32
    f32r = mybir.dt.float32r
    AF = mybir.ActivationFunctionType

    nc = bacc.Bacc("TRN2", target_bir_lowering=False, debug=False, num_devices=NC)
    blob = nc.dram_tensor("blob", [2, 128, WBLOB], f32r, kind="ExternalInput").ap()
    y_out = nc.dram_tensor("y", [2, 128, QH], f32, kind="ExternalOutput").ap()
    den_out = nc.dram_tensor("den", [1, QH], f32, kind="ExternalOutput").ap()

    with tile.TileContext(nc) as tc:
        with tc.tile_pool(name="sb", bufs=1) as sb, \
             tc.tile_pool(name="pp", bufs=10) as pp, \
             tc.tile_pool(name="acc", bufs=2) as accp, \
             tc.tile_pool(name="psA", bufs=6, space="PSUM") as psA, \
             tc.tile_pool(name="psO", bufs=1, space="PSUM") as psO, \
             (tc.For_i(0, bench_iters, 1) if bench_iters else contextlib.nullcontext()):
            bl = sb.tile([128, 2, WBLOB], f32r)
            # weights+bq section first, then xT in 512-col chunks, so the
            # projection phases start as soon as their slice has landed.
            for m in range(2):
                nc.sync.dma_start(bl[:, m, WQO:WBLOB], blob[m, :, WQO:WBLOB])
            for n in range(8):
                for m in range(2):
                    nc.sync.dma_start(bl[:, m, n * 512:(n + 1) * 512],
                                      blob[m, :, n * 512:(n + 1) * 512])
            xT = bl[:, :, XO:XO + HW]
            wq = bl[:, :, WQO:WQO + C]
            wk = bl[:, :, WKO:WKO + C]
            wv = bl[:, :, WVO:WVO + C]
            wp = bl[:, :, WPO:WPO + C]
            bq = bl[:, :, BQO:BQO + 2]

            def evac(i, out, ps):
                # alternate PSUM evacuation between ACT and DVE to halve the gate
                if i % 2 == 0:
                    nc.scalar.activation(out, ps, AF.Identity)
                else:
                    nc.vector.tensor_copy(out, ps)

            # --- K^T [c(2x128), kk 4096] ---
            kTt = sb.tile([128, 2, HW], f32r)
            for n in range(8):
                for m in range(2):
                    ps = psA.tile([128, 512], f32, tag="ps", name=f"psk{m}{n}")
                    for k in range(2):
                        nc.tensor.matmul(ps, wk[:, k, m * 128:(m + 1) * 128],
                                         xT[:, k, n * 512:(n + 1) * 512],
                                         start=(k == 0), stop=(k == 1))
                    evac(m, kTt[:, m, n * 512:(n + 1) * 512], ps)

            # --- Q^T [c(2x128), q 2048] (queries are xT cols 0:QH) ---
            qTt = sb.tile([128, 2, QH], f32r)
            for n in range(QCH):
                for m in range(2):
                    ps = psA.tile([128, 512], f32, tag="ps", name=f"psq{m}{n}")
                    for k in range(2):
                        nc.tensor.matmul(ps, wq[:, k, m * 128:(m + 1) * 128],
                                         xT[:, k, n * 512:(n + 1) * 512],
                                         start=(k == 0), stop=(k == 1))
                    evac(m, qTt[:, m, n * 512:(n + 1) * 512], ps)

            # --- V [kk(32x128), c 256] ---
            vt = sb.tile([128, KT, C], f32r)
            for t in range(KT):
                ps = psA.tile([128, C], f32, tag="ps", name=f"psv{t}")
                for k in range(2):
                    nc.tensor.matmul(ps, xT[:, k, t * 128:(t + 1) * 128], wv[:, k],
                                     start=(k == 0), stop=(k == 1))
                evac(t, vt[:, t], ps)

            # --- tau[kk] = K^T.T @ (bq/16): per-kk softmax bias term ---
            pst = psA.tile([128, 2 * KT], f32, tag="ps", name="pstau")
            for t in range(KT):
                for m in range(2):
                    nc.tensor.matmul(pst[:, 2 * t:2 * t + 2], kTt[:, m, t * 128:(t + 1) * 128],
                                     bq[:, m], start=(m == 0), stop=(m == 1),
                                     skip_group_check=True)
            tau = sb.tile([128, 2 * KT], f32)
            nc.scalar.activation(tau, pst, AF.Identity)

            # --- attention main loop over query chunks ---
            oT = sb.tile([128, 2, QH], f32r)
            den_s = sb.tile([1, QH], f32)
            for j in range(QCH):
                po0 = psO.tile([128, 512], f32, tag="o0", name=f"po0_{j}")
                po1 = psO.tile([128, 512], f32, tag="o1", name=f"po1_{j}")
                acc0 = accp.tile([128, 512], f32, tag="a0", name=f"acc0_{j}")
                acc1 = accp.tile([128, 512], f32, tag="a1", name=f"acc1_{j}")
                for t in range(KT):
                    ps = psA.tile([128, 512], f32, tag="ps", name=f"pss{j}_{t}")
                    for m in range(2):
                        nc.tensor.matmul(ps, kTt[:, m, t * 128:(t + 1) * 128],
                                         qTt[:, m, j * 512:(j + 1) * 512],
                                         start=(m == 0), stop=(m == 1))
                    pT = pp.tile([128, 512], f32r, tag="p", name=f"pt{j}_{t}")
                    nc.scalar.activation(pT, ps, AF.Exp, bias=tau[:, 2 * t:2 * t + 1], scale=1.0)
                    nc.tensor.matmul(po0, vt[:, t, 0:128], pT,
                                     start=(t == 0), stop=(t == KT - 1),
                                     skip_group_check=True)
                    nc.tensor.matmul(po1, vt[:, t, 128:256], pT,
                                     start=(t == 0), stop=(t == KT - 1),
                                     skip_group_check=True)
                    # denominator: accumulate exp tiles on DVE (two chains)
                    acc = acc0 if t % 2 == 0 else acc1
                    if t < 2:
                        nc.vector.tensor_copy(acc, pT.bitcast(f32))
                    else:
                        nc.vector.tensor_add(acc, acc, pT.bitcast(f32))
                nc.vector.tensor_add(acc0, acc0, acc1)
                # partition-reduce 128 -> 1 on GPSIMD (fp32 adds)
                accr = accp.tile([128, 512], f32, tag="ar", name=f"accr_{j}")
                nc.gpsimd.partition_all_reduce(accr, acc0, channels=128,
                                               reduce_op=bass_isa.ReduceOp.add)
                nc.vector.tensor_copy(den_s[0:1, j * 512:(j + 1) * 512], accr[0:1])
                evac(0, oT[:, 0, j * 512:(j + 1) * 512], po0)
                evac(1, oT[:, 1, j * 512:(j + 1) * 512], po1)

            # --- final projection Y^T = wp.T @ O'^T ---
            y_s = sb.tile([128, 2, QH], f32)
            for n in range(QCH):
                for m in range(2):
                    ps = psA.tile([128, 512], f32, tag="ps", name=f"psy{m}{n}")
                    for k in range(2):
                        nc.tensor.matmul(ps, wp[:, k, m * 128:(m + 1) * 128],
                                         oT[:, k, n * 512:(n + 1) * 512],
                                         start=(k == 0), stop=(k == 1))
                    evac(m, y_s[:, m, n * 512:(n + 1) * 512], ps)
            nc.sync.dma_start(y_out[0], y_s[:, 0])
            nc.sync.dma_start(y_out[1], y_s[:, 1])
            nc.sync.dma_start(den_out, den_s)
    nc.compile()
    return nc


def make_in_maps(x, wq, bq, wk, wv, wp):
    """Per-core input blobs. x: [B,H,W,C] float32."""
    xf = np.ascontiguousarray(x, np.float32).reshape(B, HW, C)
    wqs = np.ascontiguousarray(wq, np.float32) / 16.0
    bqs = np.ascontiguousarray(bq, np.float32) / 16.0
    in_maps = []
    for c in range(NC):
        b, h = divmod(c, 2)
        X = xf[b]
        xP = np.concatenate([X[h * QH:(h + 1) * QH], X[(1 - h) * QH:(2 - h) * QH]], axis=0)
        blob = np.empty((C, WBLOB), np.float32)
        blob[:, XO:XO + HW] = xP.T
        blob[:, WQO:WQO + C] = wqs
        blob[:, WKO:WKO + C] = np.asarray(wk, np.float32)
        blob[:, WVO:WVO + C] = np.asarray(wv, np.float32)
        blob[:, WPO:WPO + C] = np.asarray(wp, np.float32)
        blob[:, BQO] = bqs
        blob[:, BQO + 1] = bqs
        in_maps.append({"blob": tf32_round(blob.reshape(2, 128, WBLOB))})
    return in_maps


def postprocess(results, x, bq, bk, bv, bp, wp):
    """Assemble full output from per-core Y^T + denominators."""
    xf = np.ascontiguousarray(x, np.float32).reshape(B, HW, C)
    bvp = (np.asarray(bv, np.float64) @ np.asarray(wp, np.float64) +
           np.asarray(bp, np.float64)).astype(np.float32)
    out = np.empty((B, HW, C), np.float32)
    for c in range(NC):
        b, h = divmod(c, 2)
        yT = results[c]["y"].reshape(C, QH)          # [256, 2048]
        den = results[c]["den"].reshape(QH)          # [2048]
        rows = yT.T / den[:, None] + bvp[None, :] + xf[b, h * QH:(h + 1) * QH]
        out[b, h * QH:(h + 1) * QH] = rows
    return out.reshape(B, HH, WW, C)


_NC_CACHE = None


def _get_nc():
    global _NC_CACHE
    if _NC_CACHE is None:
        _NC_CACHE = build_nc()
    return _NC_CACHE


def kernel(x, t, wq, bq, wk, bk, wv, bv, wp, bp):
    from concourse.bass_utils import run_bass_kernel_spmd
    in_maps = make_in_maps(x, wq, bq, wk, wv, wp)
    nc = _get_nc()
    res = run_bass_kernel_spmd(nc, in_maps, core_ids=list(range(NC)))
    return postprocess(res.results, x, bq, bk, bv, bp, wp)


# revision 15
# speedup vs baseline: 512.0509x; 512.0509x over previous
"""Trainium2 Bass kernel for nn_AttentionBlock (B=4, H=W=64, C=256), SPMD over 8 NeuronCores.

Strategy:
  - Shard: batch b = core//2, query-half = core%2 (2048 queries/core, full 4096 keys).
    Key order is permuted per-core (own-half first) — softmax is permutation-invariant.
  - All matmuls in float32r (tf32): host pre-rounds inputs; PE multiplies exactly with
    fp32 PSUM accumulation (verified: only error source is the tf32 input rounding).
  - Transposed dataflow: K^T/Q^T [c, pos]; scores computed as S^T [kk, q] so softmax's
    key-reduction becomes a PE ones-matmul; exp via ACT with per-partition bias
    tau[kk] = <bq/16, K[kk]> (the only part of the q/k biases softmax doesn't cancel).
  - V natural layout [kk, c] feeds attn@V directly: O'^T[c,q] accumulated over 32 kk tiles.
  - Device returns Y^T = wp.T @ O'^T (unnormalized) + denominators; host divides,
    adds bv@wp + bp and the residual x.
"""
import numpy as np

B, HH, WW, C = 4, 64, 64, 256
HW = HH * WW          # 4096 spatial positions
QH = HW // 2          # 2048 queries per core
NC = 8
KT = HW // 128        # 32 kk tiles
QCH = QH // 512       # 4 query chunks of 512
# blob columns: xT | wq/16 | wk | wv | wp | bq/16
XO, WQO, WKO, WVO, WPO, BQO = 0, HW, HW + C, HW + 2 * C, HW + 3 * C, HW + 4 * C
WBLOB = HW + 4 * C + 2  # 5122 (bq stored twice: fp32r matmul needs even N)


def tf32_round(x: np.ndarray) -> np.ndarray:
    u = np.ascontiguousarray(x, np.float32).view(np.uint32).astype(np.uint64)
    u = (u + 0x1000 + ((u >> 13) & 1)) & 0xFFFFE000
    return u.astype(np.uint32).view(np.float32)


def build_nc(bench_iters=None, with_tau=False, qch=QCH, pair=False, den_mode="dve", lag=10):
    import contextlib
    import concourse.bass as bass  # noqa: F401
    import concourse.tile as tile
    from concourse import bacc, mybir
    from concourse import bass_isa

    f32 = mybir.dt.float32
    f32r = mybir.dt.float32r
    AF = mybir.ActivationFunctionType

    nc = bacc.Bacc("TRN2", target_bir_lowering=False, debug=False, num_devices=NC)
    blob = nc.dram_tensor("blob", [2, 128, WBLOB], f32r, kind="ExternalInput").ap()
    y_out = nc.dram_tensor("y", [2, 128, QH], f32, kind="ExternalOutput").ap()
    den_out = nc.dram_tensor("den", [1, QH], f32, kind="ExternalOutput").ap()

    psa_bufs = (4 if pair else 6) - (1 if den_mode == "pe" else 0)
    with tile.TileContext(nc) as tc:
        with contextlib.ExitStack() as _st:
            sb = _st.enter_context(tc.tile_pool(name="sb", bufs=1))
            pp = _st.enter_context(tc.tile_pool(name="pp", bufs=max(12, lag + 4)))
            accp = _st.enter_context(tc.tile_pool(name="acc", bufs=2))
            psA = _st.enter_context(tc.tile_pool(name="psA", bufs=psa_bufs, space="PSUM"))
            psO = _st.enter_context(tc.tile_pool(name="psO", bufs=(2 if pair else 1), space="PSUM"))
            psD = _st.enter_context(tc.tile_pool(name="psD", bufs=1, space="PSUM")) if den_mode == "pe" else None
            if bench_iters:
                _st.enter_context(tc.For_i(0, bench_iters, 1))
            bl = sb.tile([128, 2, WBLOB], f32r)
            # weights+bq section first, then xT in 512-col chunks, so the
            # projection phases start as soon as their slice has landed.
            for m in range(2):
                nc.sync.dma_start(bl[:, m, WQO:WBLOB], blob[m, :, WQO:WBLOB])
            for n in range(8):
                for m in range(2):
                    nc.sync.dma_start(bl[:, m, n * 512:(n + 1) * 512],
                                      blob[m, :, n * 512:(n + 1) * 512])
            xT = bl[:, :, XO:XO + HW]
            wq = bl[:, :, WQO:WQO + C]
            wk = bl[:, :, WKO:WKO + C]
            wv = bl[:, :, WVO:WVO + C]
            wp = bl[:, :, WPO:WPO + C]
            bq = bl[:, :, BQO:BQO + 2]

            def evac(i, out, ps):
                # alternate PSUM evacuation between ACT and DVE to halve the gate
                if i % 2 == 0:
                    nc.scalar.activation(out, ps, AF.Identity)
                else:
                    nc.vector.tensor_copy(out, ps)

            # --- K^T [c(2x128), kk 4096] ---
            kTt = sb.tile([128, 2, HW], f32r)
            for n in range(8):
                for m in range(2):
                    ps = psA.tile([128, 512], f32, tag="ps", name=f"psk{m}{n}")
                    for k in range(2):
                        nc.tensor.matmul(ps, wk[:, k, m * 128:(m + 1) * 128],
                                         xT[:, k, n * 512:(n + 1) * 512],
                                         start=(k == 0), stop=(k == 1))
                    evac(m, kTt[:, m, n * 512:(n + 1) * 512], ps)

            # --- Q^T [c(2x128), q 2048] (queries are xT cols 0:QH) ---
            qTt = sb.tile([128, 2, QH], f32r)
            for n in range(QCH):
                for m in range(2):
                    ps = psA.tile([128, 512], f32, tag="ps", name=f"psq{m}{n}")
                    for k in range(2):
                        nc.tensor.matmul(ps, wq[:, k, m * 128:(m + 1) * 128],
                                         xT[:, k, n * 512:(n + 1) * 512],
                                         start=(k == 0), stop=(k == 1))
                    evac(m, qTt[:, m, n * 512:(n + 1) * 512], ps)

            # --- V [kk(32x128), c 256] (emitted inside main-loop j=0) ---
            vt = sb.tile([128, KT, C], f32r)

            tau = None
            if with_tau:
                # --- tau[kk] = K^T.T @ (bq/16): per-kk softmax bias term ---
                pst = psA.tile([128, 2 * KT], f32, tag="ps", name="pstau")
                for t in range(KT):
                    for m in range(2):
                        nc.tensor.matmul(pst[:, 2 * t:2 * t + 2], kTt[:, m, t * 128:(t + 1) * 128],
                                         bq[:, m], start=(m == 0), stop=(m == 1),
                                         skip_group_check=True)
                tau = sb.tile([128, 2 * KT], f32)
                nc.scalar.activation(tau, pst, AF.Identity)

            ones_t = None
            if den_mode == "pe":
                ones_t = sb.tile([128, 2], f32r)
                nc.scalar.activation(ones_t, bl[:, :, 0:1], AF.Identity, scale=0.0, bias=1.0)

            # --- attention main loop over query chunks ---
            oT = sb.tile([128, 2, QH], f32r)
            den_s = sb.tile([1, QH], f32)
            LAG = lag
            state = {}  # j -> (po0, po1, acc0, acc1)
            pts = {}    # (j, t) -> pT tile

            def emit_consume(j, t):
                if t == 0:
                    state[j] = (
                        psO.tile([128, 512], f32, tag="o0", name=f"po0_{j}"),
                        psO.tile([128, 512], f32, tag="o1", name=f"po1_{j}"),
                        accp.tile([128, 512], f32, tag="a0", name=f"acc0_{j}")
                        if den_mode == "dve" else None,
                        accp.tile([128, 512], f32, tag="a1", name=f"acc1_{j}")
                        if den_mode == "dve" else None,
                        psD.tile([1, 512], f32, tag="d", name=f"pd_{j}")
                        if den_mode == "pe" else None,
                    )
                po0, po1, acc0, acc1, pd = state[j]
                pT = pts.pop((j, t))
                nc.tensor.matmul(po0, vt[:, t, 0:128], pT,
                                 start=(t == 0), stop=(t == KT - 1),
                                 skip_group_check=True)
                nc.tensor.matmul(po1, vt[:, t, 128:256], pT,
                                 start=(t == 0), stop=(t == KT - 1),
                                 skip_group_check=True)
                if den_mode == "dve":
                    # denominator: accumulate exp tiles on DVE (two chains)
                    acc = acc0 if t % 2 == 0 else acc1
                    if t < 2:
                        nc.vector.tensor_copy(acc, pT.bitcast(f32))
                    else:
                        nc.vector.tensor_add(acc, acc, pT.bitcast(f32))
                elif den_mode == "pe":
                    nc.tensor.matmul(pd[0:1], ones_t[:, 0:1], pT,
                                     start=(t == 0), stop=(t == KT - 1),
                                     skip_group_check=True)
                if t == KT - 1:
                    if den_mode == "dve":
                        nc.vector.tensor_add(acc0, acc0, acc1)
                        # partition-reduce 128 -> 1 on GPSIMD (fp32 adds)
                        accr = accp.tile([128, 512], f32, tag="ar", name=f"accr_{j}")
                        nc.gpsimd.partition_all_reduce(accr, acc0, channels=128,
                                                       reduce_op=bass_isa.ReduceOp.add)
                        nc.vector.tensor_copy(den_s[0:1, j * 512:(j + 1) * 512], accr[0:1])
                    elif den_mode == "pe":
                        nc.scalar.activation(den_s[0:1, j * 512:(j + 1) * 512], pd[0:1], AF.Identity)
                    else:
                        nc.vector.memset(den_s[0:1, j * 512:(j + 1) * 512], 1.0)
                    evac(0, oT[:, 0, j * 512:(j + 1) * 512], po0)
                    evac(1, oT[:, 1, j * 512:(j + 1) * 512], po1)

            # chunk pairs interleaved per t-step: denser PE work between sync points
            steps = []
            if pair:
                for jp in range(0, qch, 2):
                    sub = [jp] if jp + 1 >= qch else [jp, jp + 1]
                    for t in range(KT):
                        for j in sub:
                            steps.append((j, t))
            else:
                steps = [(j, t) for j in range(qch) for t in range(KT)]
            for idx, (j, t) in enumerate(steps):
                if j == 0:
                    # fused V projection for kk-tile t
                    psv = psA.tile([128, C], f32, tag="ps", name=f"psv{t}")
                    for k in range(2):
                        nc.tensor.matmul(psv, xT[:, k, t * 128:(t + 1) * 128], wv[:, k],
                                         start=(k == 0), stop=(k == 1))
                    evac(t, vt[:, t], psv)
                ps = psA.tile([128, 512], f32, tag="ps", name=f"pss{j}_{t}")
                for m in range(2):
                    nc.tensor.matmul(ps, kTt[:, m, t * 128:(t + 1) * 128],
                                     qTt[:, m, j * 512:(j + 1) * 512],
                                     start=(m == 0), stop=(m == 1))
                pT = pp.tile([128, 512], f32r, tag="p", name=f"pt{j}_{t}")
                if with_tau:
                    nc.scalar.activation(pT, ps, AF.Exp, bias=tau[:, 2 * t:2 * t + 1], scale=1.0)
                else:
                    nc.scalar.activation(pT, ps, AF.Exp)
                pts[(j, t)] = pT
                if idx >= LAG:
                    emit_consume(*steps[idx - LAG])
            for idx in range(len(steps) - LAG, len(steps)):
                emit_consume(*steps[idx])

            # --- final projection Y^T = wp.T @ O'^T ---
            y_s = sb.tile([128, 2, QH], f32)
            for n in range(qch):
                for m in range(2):
                    ps = psA.tile([128, 512], f32, tag="ps", name=f"psy{m}{n}")
                    for k in range(2):
                        nc.tensor.matmul(ps, wp[:, k, m * 128:(m + 1) * 128],
                                         oT[:, k, n * 512:(n + 1) * 512],
                                         start=(k == 0), stop=(k == 1))
                    evac(m, y_s[:, m, n * 512:(n + 1) * 512], ps)
            nc.sync.dma_start(y_out[0], y_s[:, 0])
            nc.sync.dma_start(y_out[1], y_s[:, 1])
            nc.sync.dma_start(den_out, den_s)
    nc.compile()
    return nc


def make_in_maps(x, wq, bq, wk, wv, wp):
    """Per-core input blobs. x: [B,H,W,C] float32."""
    xf = np.ascontiguousarray(x, np.float32).reshape(B, HW, C)
    wqs = np.ascontiguousarray(wq, np.float32) / 16.0
    bqs = np.ascontiguousarray(bq, np.float32) / 16.0
    in_maps = []
    for c in range(NC):
        b, h = divmod(c, 2)
        X = xf[b]
        xP = np.concatenate([X[h * QH:(h + 1) * QH], X[(1 - h) * QH:(2 - h) * QH]], axis=0)
        blob = np.empty((C, WBLOB), np.float32)
        blob[:, XO:XO + HW] = xP.T
        blob[:, WQO:WQO + C] = wqs
        blob[:, WKO:WKO + C] = np.asarray(wk, np.float32)
        blob[:, WVO:WVO + C] = np.asarray(wv, np.float32)
        blob[:, WPO:WPO + C] = np.asarray(wp, np.float32)
        blob[:, BQO] = bqs
        blob[:, BQO + 1] = bqs
        in_maps.append({"blob": tf32_round(blob.reshape(2, 128, WBLOB))})
    return in_maps


def postprocess(results, x, bq, bk, bv, bp, wp):
    """Assemble full output from per-core Y^T + denominators."""
    xf = np.ascontiguousarray(x, np.float32).reshape(B, HW, C)
    bvp = (np.asarray(bv, np.float64) @ np.asarray(wp, np.float64) +
           np.asarray(bp, np.float64)).astype(np.float32)
    out = np.empty((B, HW, C), np.float32)
    for c in range(NC):
        b, h = divmod(c, 2)
        yT = results[c]["y"].reshape(C, QH)          # [256, 2048]
        den = results[c]["den"].reshape(QH)          # [2048]
        rows = yT.T / den[:, None] + bvp[None, :] + xf[b, h * QH:(h + 1) * QH]
        out[b, h * QH:(h + 1) * QH] = rows
    return out.reshape(B, HH, WW, C)


_NC_CACHE = {}


def _get_nc(with_tau=False):
    if with_tau not in _NC_CACHE:
        _NC_CACHE[with_tau] = build_nc(with_tau=with_tau)
    return _NC_CACHE[with_tau]


def kernel(x, t, wq, bq, wk, bk, wv, bv, wp, bp):
    from concourse.bass_utils import run_bass_kernel_spmd
    in_maps = make_in_maps(x, wq, bq, wk, wv, wp)
    with_tau = bool(np.any(np.asarray(bq)))
    nc = _get_nc(with_tau)
    res = run_bass_kernel_spmd(nc, in_maps, core_ids=list(range(NC)))
    return postprocess(res.results, x, bq, bk, bv, bp, wp)
